# revision 9
# baseline (speedup 1.0000x reference)
"""Trainium2 Bass kernel for nn_MoE_27255862461168 (moe_routing).

Math notes (derived from the reference):
- With TOPK=1 and no ties (verified for the fixed seed), the faithful
  masked_scatter_ semantics reduce to output[n] = x.flat[n], where x is the
  [N_exp, E] dense expert output matrix — so only the first ceil(N/E)=10923
  rows of expert outputs are needed.  The gate only affects kld_loss.
- SIREN sin(w0*z) computed via range reduction: fold s=w0/(2pi) into the
  weights so the matmul produces t=z/(2pi); r=rne(t) via the (t+1.5*2^23)
  -1.5*2^23 trick on DVE; frac=t-r accumulated on PE via a -I matmul into
  the same PSUM tile; ACT Sin(scale~2pi) gives sin(z) exactly (periodicity).
- Biases ride as an extra contraction row (rhs row pinned to 1.0); the
  constant-one row of each activation tile is produced by an extra weight
  column (bias 0.25 -> sin(pi/2)=1).
"""
import sys
sys.path.insert(0, '/opt/trn_rl_repo')
import numpy as np
from contextlib import ExitStack

import concourse.bass as bass
import concourse.tile as tile
from concourse import bacc, mybir
from concourse.bass_utils import run_bass_kernel_spmd
from concourse.tile_rust import add_dep_helper

N = 65536
CC = 3
GF = 16
E = 6
F = 255
HID = 3
BW = 45.0
LN_EPS = 1e-5
NCORES = 8
ME = -(-N // E)            # 10923 expert tokens needed
NE = -(-ME // NCORES)      # 1366 expert tokens per core
NG = N // NCORES           # 8192 gate tokens per core
GB = 8                     # gate bands packed on partitions
GT = NG // GB              # 1024 gate tokens per band

F32 = mybir.dt.float32
A = mybir.AluOpType
AF = mybir.ActivationFunctionType
MAGIC = 12582912.0         # 1.5*2^23: (t+MAGIC)-MAGIC = round-to-nearest-even
TWO_PI = 6.2831845         # fp32 just under 2*pi (keeps ACT Sin arg inside [-pi,pi])
CONSTC = 0.25              # const-col bias: frac=0.25 -> sin(pi/2)=1.0


def _chunks(total, step=512):
    out = []
    off = 0
    while off < total:
        sz = min(step, total - off)
        out.append((off, sz))
        off += sz
    return out


def build_program(ne=NE, gt=GT, n_gate_chunks=None):
    """Build the single-core SPMD program (same on all 8 cores)."""
    nc = bacc.Bacc('TRN2', target_bir_lowering=False, debug=False,
                   num_devices=1)
    dp = nc.declare_dram_parameter
    i_ec = dp("ec", [4, ne], F32, isOutput=False)
    i_gc = dp("gc", [25, gt], F32, isOutput=False)
    i_negI = dp("negI", [128, 128], F32, isOutput=False)
    i_wfx = dp("wfx", [4, GF], F32, isOutput=False)
    i_wfbd = dp("wfbd", [25, 128], F32, isOutput=False)
    i_g0bd = dp("g0bd", [128, 128], F32, isOutput=False)
    i_g1bd = dp("g1bd", [128, 128], F32, isOutput=False)
    i_g2bd = dp("g2bd", [128, 128], F32, isOutput=False)
    i_g3bd = dp("g3bd", [128, 48], F32, isOutput=False)
    i_mubd = dp("mubd", [128, 8], F32, isOutput=False)
    i_bc16 = dp("bc16", [8, 128], F32, isOutput=False)
    i_ones6 = dp("ones6", [48, 8], F32, isOutput=False)
    i_bc6 = dp("bc6", [8, 48], F32, isOutput=False)
    i_gb0t = dp("gb0t", [128, 1], F32, isOutput=False)
    i_gb1t = dp("gb1t", [128, 1], F32, isOutput=False)
    i_gb2t = dp("gb2t", [128, 1], F32, isOutput=False)
    i_gb3t = dp("gb3t", [48, 1], F32, isOutput=False)
    i_lngt = dp("lngt", [128, 1], F32, isOutput=False)
    i_lnbt = dp("lnbt", [128, 1], F32, isOutput=False)
    i_w0x = dp("w0x", [E, 17, 256], F32, isOutput=False)
    i_whk0 = dp("whk0", [E, HID, 128, 256], F32, isOutput=False)
    i_whk1 = dp("whk1", [E, HID, 128, 256], F32, isOutput=False)
    i_wok0 = dp("wok0", [E, 128, 6], F32, isOutput=False)
    i_wok1 = dp("wok1", [E, 128, 6], F32, isOutput=False)
    o_xo = dp("xo", [6, ne], F32, isOutput=True)
    o_mo = dp("mo", [48, 1], F32, isOutput=True)

    ech = _chunks(ne)
    gch = _chunks(gt)
    if n_gate_chunks is not None:
        gch = gch[:n_gate_chunks]

    with tile.TileContext(nc) as tc, ExitStack() as ctx:
        wp = ctx.enter_context(tc.tile_pool(name="w", bufs=1))
        gp = ctx.enter_context(tc.tile_pool(name="g", bufs=1))
        ep = ctx.enter_context(tc.tile_pool(name="e", bufs=1))
        yp = ctx.enter_context(tc.tile_pool(name="y", bufs=3))
        rp = ctx.enter_context(tc.tile_pool(name="r", bufs=3))
        xp = ctx.enter_context(tc.tile_pool(name="x", bufs=2))
        pz = ctx.enter_context(tc.tile_pool(name="pz", bufs=5, space="PSUM"))
        pg = ctx.enter_context(tc.tile_pool(name="pg", bufs=1, space="PSUM"))
        px = ctx.enter_context(tc.tile_pool(name="px", bufs=1, space="PSUM"))

        def wtile(name, src_ap, shape):
            t = wp.tile(shape, F32, name=name)
            nc.sync.dma_start(t[:], src_ap)
            return t

        # ---- weights ----
        t_negI = wtile("negI", i_negI[:], [128, 128])
        t_wfx = wtile("wfx", i_wfx[:], [4, GF])
        t_w0 = [wtile(f"w0_{e}", i_w0x[e], [17, 256]) for e in range(E)]
        t_wh = [[(wtile(f"whk0_{e}_{l}", i_whk0[e, l], [128, 256]),
                  wtile(f"whk1_{e}_{l}", i_whk1[e, l], [128, 256]))
                 for l in range(HID)] for e in range(E)]
        t_wo = [(wtile(f"wok0_{e}", i_wok0[e], [128, 6]),
                 wtile(f"wok1_{e}", i_wok1[e], [128, 6])) for e in range(E)]
        t_wfbd = wtile("wfbd", i_wfbd[:], [25, 128])
        t_g0bd = wtile("g0bd", i_g0bd[:], [128, 128])
        t_g1bd = wtile("g1bd", i_g1bd[:], [128, 128])
        t_g2bd = wtile("g2bd", i_g2bd[:], [128, 128])
        t_g3bd = wtile("g3bd", i_g3bd[:], [128, 48])
        t_mubd = wtile("mubd", i_mubd[:], [128, 8])
        t_bc16 = wtile("bc16", i_bc16[:], [8, 128])
        t_ones6 = wtile("ones6", i_ones6[:], [48, 8])
        t_bc6 = wtile("bc6", i_bc6[:], [8, 48])
        t_gb0t = wtile("gb0t", i_gb0t[:], [128, 1])
        t_gb1t = wtile("gb1t", i_gb1t[:], [128, 1])
        t_gb2t = wtile("gb2t", i_gb2t[:], [128, 1])
        t_gb3t = wtile("gb3t", i_gb3t[:], [48, 1])
        t_lngt = wtile("lngt", i_lngt[:], [128, 1])
        t_lnbt = wtile("lnbt", i_lnbt[:], [128, 1])
        t_epsb = wp.tile([8, 1], F32, name="epsb")
        nc.gpsimd.memset(t_epsb[:], LN_EPS)

        # ---- expert feature: featT [17, ne], row 16 = ones ----
        t_ec = ep.tile([4, ne], F32, name="ec")
        nc.sync.dma_start(t_ec[:], i_ec[:])
        featT = ep.tile([17, ne], F32, name="featT")
        nc.gpsimd.memset(featT[:, :], 1.0)
        for off, sz in ech:
            p_f = pg.tile([16, 512], F32, name="pgA")[:, :sz]
            nc.tensor.matmul(p_f, t_wfx[:], t_ec[:, off:off + sz],
                             start=True, stop=True)
            nc.vector.tensor_copy(featT[0:16, off:off + sz], p_f)

        # ---- gate (all ACT ops use exp/ln/relu/copy: one table set) ----
        t_gc = gp.tile([25, gt], F32, name="gc")
        nc.sync.dma_start(t_gc[:], i_gc[:])
        t_m48 = gp.tile([48, len(gch)], F32, name="m48")
        last_gate_act = None
        for gi, (off, sz) in enumerate(gch):
            sl = slice(off, off + sz)
            p = pg.tile([128, 512], F32, name="pgA")[:, :sz]
            nc.tensor.matmul(p, t_wfbd[:], t_gc[:, sl], start=True, stop=True)
            ft = gp.tile([128, 512], F32, name="ft")[:, :sz]
            nc.scalar.copy(ft, p)
            p = pg.tile([128, 512], F32, name="pgA")[:, :sz]
            nc.tensor.matmul(p, t_g0bd[:], ft, start=True, stop=True)
            h0 = gp.tile([128, 512], F32, name="h0")[:, :sz]
            nc.scalar.activation(h0, p, AF.Relu, bias=t_gb0t[:])
            p = pg.tile([128, 512], F32, name="pgA")[:, :sz]
            nc.tensor.matmul(p, t_g1bd[:], h0, start=True, stop=True)
            h1 = gp.tile([128, 512], F32, name="h1")[:, :sz]
            nc.scalar.activation(h1, p, AF.Relu, bias=t_gb1t[:])
            p = pg.tile([128, 512], F32, name="pgA")[:, :sz]
            nc.tensor.matmul(p, t_g2bd[:], h1, start=True, stop=True)
            h2 = gp.tile([128, 512], F32, name="h2")[:, :sz]
            nc.vector.tensor_scalar(h2, p, t_gb2t[:], None, op0=A.add)
            pmu = pg.tile([8, 512], F32, name="pgA")[:, :sz]
            nc.tensor.matmul(pmu, t_mubd[:], h2, start=True, stop=True)
            mus = gp.tile([8, 512], F32, name="mus")[:, :sz]
            nc.vector.tensor_copy(mus, pmu)
            pmb = pg.tile([128, 512], F32, name="pgA")[:, :sz]
            nc.tensor.matmul(pmb, t_bc16[:], mus, start=True, stop=True)
            hc = gp.tile([128, 512], F32, name="hc")[:, :sz]
            nc.vector.tensor_tensor(hc, h2, pmb, op=A.subtract)
            hc2 = gp.tile([128, 512], F32, name="hc2")[:, :sz]
            nc.vector.tensor_tensor(hc2, hc, hc, op=A.mult)
            pvar = pg.tile([8, 512], F32, name="pgA")[:, :sz]
            nc.tensor.matmul(pvar, t_mubd[:], hc2, start=True, stop=True)
            lnv = gp.tile([8, 512], F32, name="lnv")[:, :sz]
            nc.scalar.activation(lnv, pvar, AF.Ln, bias=t_epsb[:])
            rstd = gp.tile([8, 512], F32, name="rstd")[:, :sz]
            nc.scalar.activation(rstd, lnv, AF.Exp, scale=-0.5)
            prs = pg.tile([128, 512], F32, name="pgA")[:, :sz]
            nc.tensor.matmul(prs, t_bc16[:], rstd, start=True, stop=True)
            hn = gp.tile([128, 512], F32, name="hn")[:, :sz]
            nc.vector.tensor_tensor(hn, hc, prs, op=A.mult)
            hna = gp.tile([128, 512], F32, name="hna")[:, :sz]
            nc.vector.tensor_scalar(hna, hn, t_lngt[:], t_lnbt[:],
                                    op0=A.mult, op1=A.add)
            plg = pg.tile([48, 512], F32, name="pgA")[:, :sz]
            nc.tensor.matmul(plg, t_g3bd[:], hna, start=True, stop=True)
            eb = gp.tile([48, 512], F32, name="eb")[:, :sz]
            nc.scalar.activation(eb, plg, AF.Exp, bias=t_gb3t[:])
            ps6 = pg.tile([8, 512], F32, name="pgA")[:, :sz]
            nc.tensor.matmul(ps6, t_ones6[:], eb, start=True, stop=True)
            lns = gp.tile([8, 512], F32, name="lns")[:, :sz]
            nc.scalar.activation(lns, ps6, AF.Ln)
            pl6 = pg.tile([48, 512], F32, name="pgA")[:, :sz]
            nc.tensor.matmul(pl6, t_bc6[:], lns, start=True, stop=True)
            rb = gp.tile([48, 512], F32, name="rb")[:, :sz]
            last_gate_act = nc.scalar.activation(rb, pl6, AF.Exp, scale=-1.0)
            scr = gp.tile([48, 512], F32, name="scr")[:, :sz]
            nc.vector.tensor_tensor(scr, eb, rb, op=A.mult)
            nc.vector.reduce_sum(t_m48[:, gi:gi + 1], scr,
                                 axis=mybir.AxisListType.X)
        t_mo = gp.tile([48, 1], F32, name="mo")
        nc.vector.reduce_sum(t_mo[:], t_m48[:], axis=mybir.AxisListType.X)
        nc.sync.dma_start(o_mo[:], t_mo[:])

        # ---- experts: nchunk outer, expert inner ----
        first_sin = [None]
        for off, sz in ech:
            sl = slice(off, off + sz)
            p_x = px.tile([6, 512], F32, name="px")[:, :sz]
            for e in range(E):
                y = None
                for l in range(1 + HID):
                    y2 = yp.tile([128, 2, 512], F32, name="y")[:, :, :sz]
                    for m in (0, 1):
                        msl = slice(m * 128, (m + 1) * 128)
                        p_t = pz.tile([128, 512], F32, name="pz")[:, :sz]
                        if l == 0:
                            nc.tensor.matmul(p_t, t_w0[e][:, msl],
                                             featT[:, sl],
                                             start=True, stop=True)
                        else:
                            nc.tensor.matmul(p_t, t_wh[e][l - 1][0][:, msl],
                                             y[:, 0, :], start=True,
                                             stop=False)
                            nc.tensor.matmul(p_t, t_wh[e][l - 1][1][:, msl],
                                             y[:, 1, :], start=False,
                                             stop=True)
                        t_r = rp.tile([128, 512], F32, name="r")[:, :sz]
                        nc.vector.tensor_scalar(t_r, p_t, MAGIC, MAGIC,
                                                op0=A.add, op1=A.subtract)
                        nc.tensor.matmul(p_t, t_negI[:], t_r, start=False,
                                         stop=True, skip_group_check=True)
                        sin_i = nc.scalar.activation(y2[:, m, :], p_t,
                                                     AF.Sin, scale=TWO_PI)
                        if first_sin[0] is None:
                            first_sin[0] = sin_i
                            add_dep_helper(
                                sin_i.ins, last_gate_act.ins, sync=False,
                                reason="gate ACT table set before Sin set")
                    y = y2
                nc.tensor.matmul(p_x, t_wo[e][0][:], y[:, 0, :],
                                 start=(e == 0), stop=False,
                                 skip_group_check=True)
                nc.tensor.matmul(p_x, t_wo[e][1][:], y[:, 1, :],
                                 start=False, stop=(e == E - 1),
                                 skip_group_check=True)
            xt = xp.tile([6, 512], F32, name="xt")[:, :sz]
            nc.vector.tensor_copy(xt, p_x)
            nc.sync.dma_start(o_xo[:, sl], xt)
    nc.compile()
    return nc


def _blockdiag(block, nb):
    br, bc = block.shape
    out = np.zeros((br * nb, bc * nb), np.float64)
    for i in range(nb):
        out[i * br:(i + 1) * br, i * bc:(i + 1) * bc] = block
    return out


def prep_weights(inp):
    """Host-side packing of weights (f64 intermediate, f32 output)."""
    f = {k: np.asarray(v, np.float64) for k, v in inp.items()}
    w0s = BW / 2.0 + np.arange(E) * BW
    s = w0s / (2.0 * np.pi)

    w0x = np.zeros((E, 17, 256), np.float64)
    whk = np.zeros((E, HID, 256, 256), np.float64)
    wox = np.zeros((E, 256, 6), np.float64)
    for e in range(E):
        w0x[e, :GF, :F] = s[e] * f['We0'][e]
        w0x[e, GF, :F] = s[e] * f['be0'][e]
        w0x[e, GF, 255] = CONSTC
        for l in range(HID):
            whk[e, l, :F, :F] = s[e] * f['Weh'][e][l]
            whk[e, l, 255, :F] = s[e] * f['beh'][e][l]
            whk[e, l, 255, 255] = CONSTC
        wox[e, :F, e] = f['Weo'][e][:, 0]
        wox[e, 255, e] = f['beo'][e][0]

    wfbd = np.zeros((25, 128), np.float64)
    for b in range(GB):
        wfbd[3 * b:3 * b + 3, GF * b:GF * (b + 1)] = f['Wf']
        wfbd[24, GF * b:GF * (b + 1)] = f['bf']

    d = {
        'negI': -np.eye(128),
        'wfx': np.vstack([f['Wf'], f['bf'][None, :]]),
        'wfbd': wfbd,
        'g0bd': _blockdiag(f['gW0'], GB),
        'g1bd': _blockdiag(f['gW1'], GB),
        'g2bd': _blockdiag(f['gW2'], GB),
        'g3bd': _blockdiag(f['gW3'], GB),
        'mubd': _blockdiag(np.full((GF, 1), 1.0 / GF), GB),
        'bc16': _blockdiag(np.ones((1, GF)), GB),
        'ones6': _blockdiag(np.ones((E, 1)), GB),
        'bc6': _blockdiag(np.ones((1, E)), GB),
        'gb0t': np.tile(f['gb0'], GB)[:, None],
        'gb1t': np.tile(f['gb1'], GB)[:, None],
        'gb2t': np.tile(f['gb2'], GB)[:, None],
        'gb3t': np.tile(f['gb3'], GB)[:, None],
        'lngt': np.tile(f['ln_g'], GB)[:, None],
        'lnbt': np.tile(f['ln_b'], GB)[:, None],
        'w0x': w0x,
        'whk0': whk[:, :, 0:128, :],
        'whk1': whk[:, :, 128:256, :],
        'wok0': wox[:, 0:128, :],
        'wok1': wox[:, 128:256, :],
    }
    return {k: np.ascontiguousarray(v, np.float32) for k, v in d.items()}


def prep_core_inputs(coords, weights, core):
    """Per-core coordinate slices (band-packed gate + expert coordsT)."""
    co = np.asarray(coords, np.float32)
    gsl = co[core * NG:(core + 1) * NG]                       # [NG, 3]
    gc = gsl.reshape(GB, GT, CC).transpose(0, 2, 1).reshape(24, GT)
    gc = np.vstack([gc, np.ones((1, GT), np.float32)])
    e0 = core * NE
    esl = co[e0:e0 + NE]
    if esl.shape[0] < NE:                                     # pad (never hit: 8*NE<N)
        esl = np.vstack([esl, np.zeros((NE - esl.shape[0], CC), np.float32)])
    ec = np.vstack([esl.T, np.ones((1, NE), np.float32)])
    m = dict(weights)
    m['gc'] = np.ascontiguousarray(gc)
    m['ec'] = np.ascontiguousarray(ec)
    return m


_NC_CACHE = {}


def _get_program():
    key = (NE, GT)
    if key not in _NC_CACHE:
        _NC_CACHE[key] = build_program()
    return _NC_CACHE[key]


def run(inputs, trace=False):
    weights = prep_weights(inputs)
    coords = np.asarray(inputs['coords'], np.float32)
    in_maps = [prep_core_inputs(coords, weights, c) for c in range(NCORES)]
    nc = _get_program()
    res = run_bass_kernel_spmd(nc, in_maps, core_ids=list(range(NCORES)),
                               trace=trace)
    X = np.concatenate([res.results[c]['xo'] for c in range(NCORES)], axis=1)
    out = X.T.reshape(-1)[:N].reshape(N, 1).astype(np.float32)
    m48 = np.stack([res.results[c]['mo'][:, 0] for c in range(NCORES)])
    m = m48.reshape(NCORES, GB, E).sum(axis=(0, 1))
    q = N / E
    kld = float((m * np.log(m) - m * np.log(q)).sum() / N / E)
    return (out, np.float32(kld)), res


def kernel(**inputs):
    (out, kld), _ = run(inputs, trace=False)
    return out, kld
